# revision 20
# baseline (speedup 1.0000x reference)
"""Distributed GIN message-passing network on 8 Trainium2 NeuronCores.

Strategy (matches the sharding hint):
  - dst-nodes (and their incident edges) are partitioned across the 8 cores;
  - the 128x128 MLP weights are replicated;
  - per-graph pooled sums and BatchNorm statistics are all-reduced;
  - the per-layer node features are all-gathered (every core needs the full
    h table to gather edge messages from).

Per layer, each core:
  1. bulk indirect-DMA gathers h[src] rows (fp16 message copy of h) for its
     ~400k edges, 128 edges per PE chunk;
  2. segment-sums them into per-dst-block aggregates with one-hot matmuls
     accumulated in PSUM (transposed [feat, node] layout);
  3. runs the GIN MLP in fp32 (z = h + agg, two Linear+ReLU, then global
     BatchNorm with all-reduced sums/sumsqs);
  4. pools h into per-graph sums with one-hot matmuls, all-reduces them and
     runs the (replicated) fc2 graph MLP;
  5. transposes h back to row-major fp16 and all-gathers it for next layer.

All graph structure (edge lists sorted by dst block, one-hot slot ids,
padding) is precomputed on the host and shipped as per-core inputs.
"""

import sys

if "/opt/trn_rl_repo" not in sys.path:
    sys.path.insert(0, "/opt/trn_rl_repo")

import numpy as np

import concourse.bass as bass
import concourse.bacc as bacc
import concourse.mybir as mybir
import concourse.tile as tile
from concourse import bass_utils
from concourse.masks import make_identity

F32 = mybir.dt.float32
F16 = mybir.dt.float16
I32 = mybir.dt.int32
I16 = mybir.dt.int16
AF = mybir.ActivationFunctionType
ALU = mybir.AluOpType

NCORES = 8
D = 128
C = 10
BN_EPS = 1e-5
GRP_W = 448  # MLP moving-dim group width
PAD_SLOT = 255.0  # one-hot slot id for padding edges (never matches iota 0..127)
PAD_GRAPH = 30000.0  # pool slot id for padding nodes


def _bcast(ap, axis_count):
    """Broadcast a [128, 1]-ish AP along a new trailing free dim."""
    return bass.AP(ap.tensor, ap.offset, [*ap.ap, [0, axis_count]])


class Cfg:
    def __init__(self, N, E, G, nch_bc):
        assert N % NCORES == 0
        self.N, self.E, self.G = N, E, G
        self.SH = N // NCORES                       # real nodes per shard
        self.NB = (self.SH + 127) // 128            # dst blocks per shard
        self.SHP = self.NB * 128                    # padded shard rows
        self.NP = NCORES * self.SHP                 # padded global rows
        # gather-table chunking: int16 indices limit a chunk to 32767 rows
        k = min(max(32767 // self.SHP, 1), NCORES)
        self.CHR = k * self.SHP                     # rows per table chunk
        self.NCHK = (self.NP + self.CHR - 1) // self.CHR
        self.NCH_BC = nch_bc          # 128-edge chunks per (dst block, table chunk)
        self.NCH_B = self.NCHK * nch_bc             # 128-edge chunks per dst block
        self.NGRP = (self.SHP + GRP_W - 1) // GRP_W
        self.GT = (G + 127) // 128                  # graph tiles for final softmax


def build_program(cfg, stop_stage=99):
    nc = bacc.Bacc("TRN2", target_bir_lowering=False, debug=False,
                   num_devices=NCORES)
    G, NB, NCH_B, SHP, NP = cfg.G, cfg.NB, cfg.NCH_B, cfg.SHP, cfg.NP
    NCH_BC = cfg.NCH_BC
    NCOL = NB * NCH_B

    # ---- I/O ----------------------------------------------------------------
    IDXW = 8 * NCH_B                                # idx cols per dst block
    xT_d = nc.dram_tensor("xT", [D, SHP], F32, kind="ExternalInput")
    x16_d = nc.dram_tensor("x16", [NP, D], F16, kind="ExternalInput")
    idx_d = nc.dram_tensor("idx16", [128, NB * IDXW], I16, kind="ExternalInput")
    ld_d = nc.dram_tensor("ld", [128, NCOL], F16, kind="ExternalInput")
    batchv_d = nc.dram_tensor("batchv", [128, NB], F16, kind="ExternalInput")
    iota128_d = nc.dram_tensor("iota128", [128, 128], F16, kind="ExternalInput")
    iotaG_d = nc.dram_tensor("iotaG", [128, G], F16, kind="ExternalInput")
    g0T_d = nc.dram_tensor("g0T", [D, G], F32, kind="ExternalInput")
    w1_d = nc.dram_tensor("conv_W1", [5, D, D], F32, kind="ExternalInput")
    w2_d = nc.dram_tensor("conv_W2", [5, D, D], F32, kind="ExternalInput")
    b1_d = nc.dram_tensor("conv_b1", [5, D], F32, kind="ExternalInput")
    b2_d = nc.dram_tensor("conv_b2", [5, D], F32, kind="ExternalInput")
    gam_d = nc.dram_tensor("conv_gamma", [5, D], F32, kind="ExternalInput")
    bet_d = nc.dram_tensor("conv_beta", [5, D], F32, kind="ExternalInput")
    fc2w_d = nc.dram_tensor("fc2_W", [D, D], F32, kind="ExternalInput")
    fc2b_d = nc.dram_tensor("fc2_b", [D], F32, kind="ExternalInput")
    fc2g_d = nc.dram_tensor("fc2_gamma", [D], F32, kind="ExternalInput")
    fc2be_d = nc.dram_tensor("fc2_beta", [D], F32, kind="ExternalInput")
    linw_d = nc.dram_tensor("lin_W", [D, C], F32, kind="ExternalInput")
    linb_d = nc.dram_tensor("lin_b", [C], F32, kind="ExternalInput")
    out_d = nc.dram_tensor("out", [G, C], F32, kind="ExternalOutput")
    dbg_d = nc.dram_tensor("dbg", [D, 256], F32, kind="ExternalOutput")

    from contextlib import ExitStack
    with tile.TileContext(nc) as tc, ExitStack() as ctx:
        sb = ctx.enter_context(tc.tile_pool(name="sb", bufs=1))
        dram = ctx.enter_context(tc.tile_pool(name="dram", bufs=1, space="DRAM"))
        gat_p = ctx.enter_context(tc.tile_pool(name="gat", bufs=3))
        s_p = ctx.enter_context(tc.tile_pool(name="sp", bufs=3))
        small_p = ctx.enter_context(tc.tile_pool(name="small", bufs=3))
        ps_agg = ctx.enter_context(tc.tile_pool(name="ps_agg", bufs=2, space="PSUM"))
        ps_mlp = ctx.enter_context(tc.tile_pool(name="ps_mlp", bufs=1, space="PSUM"))
        ps_tr = ctx.enter_context(tc.tile_pool(name="ps_tr", bufs=2, space="PSUM"))
        ps_pool = ctx.enter_context(tc.tile_pool(name="ps_pool", bufs=1, space="PSUM"))

        # ---- persistent SBUF state ------------------------------------------
        hT = sb.tile([D, SHP], F32, tag="hT")          # current h (transposed)
        ld_t = sb.tile([128, NCOL], F16, tag="ld")
        batchv_t = sb.tile([128, NB], F16, tag="batchv")
        iota128_t = sb.tile([128, 128], F16, tag="iota128")
        iotaG_t = sb.tile([128, G], F16, tag="iotaG")
        ident_t = sb.tile([128, 128], F32, tag="ident")
        gT = sb.tile([D, G], F32, tag="gT")
        totT = sb.tile([D, G], F32, tag="totT")

        nc.sync.dma_start(out=hT[:], in_=xT_d[:, :])
        nc.sync.dma_start(out=ld_t[:], in_=ld_d[:, :])
        nc.sync.dma_start(out=batchv_t[:], in_=batchv_d[:, :])
        nc.sync.dma_start(out=iota128_t[:], in_=iota128_d[:, :])
        nc.sync.dma_start(out=iotaG_t[:], in_=iotaG_d[:, :])
        nc.sync.dma_start(out=gT[:], in_=g0T_d[:, :])
        nc.sync.dma_start(out=totT[:], in_=g0T_d[:, :])
        make_identity(nc, ident_t[:])

        # replicated weights
        w1_t, w2_t, b1_t, b2_t, gam_t, bet_t = [], [], [], [], [], []
        for i in range(5):
            w1 = sb.tile([D, D], F32, tag=f"w1_{i}")
            w2 = sb.tile([D, D], F32, tag=f"w2_{i}")
            nc.sync.dma_start(out=w1[:], in_=w1_d[i, :, :])
            nc.sync.dma_start(out=w2[:], in_=w2_d[i, :, :])
            w1_t.append(w1)
            w2_t.append(w2)
            for lst, src in ((b1_t, b1_d), (b2_t, b2_d), (gam_t, gam_d),
                             (bet_t, bet_d)):
                t = sb.tile([D, 1], F32, tag=f"v{len(lst)}_{src.name}")
                nc.sync.dma_start(out=t[:], in_=src[i, :, None])
                lst.append(t)
        fc2w_t = sb.tile([D, D], F32, tag="fc2w")
        nc.sync.dma_start(out=fc2w_t[:], in_=fc2w_d[:, :])
        fc2b_t = sb.tile([D, 1], F32, tag="fc2b")
        fc2g_t = sb.tile([D, 1], F32, tag="fc2g")
        fc2be_t = sb.tile([D, 1], F32, tag="fc2be")
        nc.sync.dma_start(out=fc2b_t[:], in_=fc2b_d[:, None])
        nc.sync.dma_start(out=fc2g_t[:], in_=fc2g_d[:, None])
        nc.sync.dma_start(out=fc2be_t[:], in_=fc2be_d[:, None])
        linw_t = sb.tile([D, C], F32, tag="linw")
        linb_t = sb.tile([C, 1], F32, tag="linb")
        nc.sync.dma_start(out=linw_t[:], in_=linw_d[:, :])
        nc.sync.dma_start(out=linb_t[:], in_=linb_d[:, None])

        # all-gathered h tables (fp16 row-major), one per producing layer
        hfull = [x16_d]
        for i in range(4):
            hfull.append(dram.tile([NP, D], F16, tag=f"hfull_{i}",
                                   name=f"hfull_{i}", addr_space="Shared"))

        rg = [list(range(NCORES))]

        def bn_stats_allreduce(stat_sb, li):
            """AR a [128, 2] stats tile across cores; returns reduced tile."""
            bn_in = dram.tile([128, 2], F32, tag=f"bn_in_{li}")
            bn_out = dram.tile([128, 2], F32, tag=f"bn_out_{li}",
                               addr_space="Shared")
            nc.sync.dma_start(out=bn_in[:], in_=stat_sb[:])
            nc.gpsimd.collective_compute(
                "AllReduce", ALU.add, replica_groups=rg,
                ins=[bn_in[:]], outs=[bn_out[:]])
            red = small_p.tile([128, 2], F32, tag="bn_red")
            nc.sync.dma_start(out=red[:], in_=bn_out[:])
            return red

        def bn_affine(red, n_count, gamma, beta, tag):
            """From AR'd [S, S2] compute scale a = gamma*rsqrt(var+eps) and
            bias b = beta - mu*a; returns (a, b) [128,1] tiles."""
            mu = small_p.tile([128, 1], F32, tag=f"mu_{tag}")
            ex2 = small_p.tile([128, 1], F32, tag=f"ex2_{tag}")
            nc.vector.tensor_scalar_mul(out=mu[:], in0=red[:, 0:1],
                                        scalar1=1.0 / n_count)
            nc.vector.tensor_scalar_mul(out=ex2[:], in0=red[:, 1:2],
                                        scalar1=1.0 / n_count)
            var = small_p.tile([128, 1], F32, tag=f"var_{tag}")
            nc.vector.tensor_tensor(out=var[:], in0=mu[:], in1=mu[:],
                                    op=ALU.mult)
            nc.vector.tensor_tensor(out=var[:], in0=ex2[:], in1=var[:],
                                    op=ALU.subtract)
            nc.vector.tensor_scalar_add(out=var[:], in0=var[:],
                                        scalar1=float(BN_EPS))
            sd = small_p.tile([128, 1], F32, tag=f"sd_{tag}")
            nc.scalar.activation(out=sd[:], in_=var[:], func=AF.Sqrt)
            rsd = small_p.tile([128, 1], F32, tag=f"rsd_{tag}")
            nc.vector.reciprocal(out=rsd[:], in_=sd[:])
            a = small_p.tile([128, 1], F32, tag=f"a_{tag}")
            nc.vector.tensor_tensor(out=a[:], in0=gamma[:], in1=rsd[:],
                                    op=ALU.mult)
            b = small_p.tile([128, 1], F32, tag=f"b_{tag}")
            nc.vector.tensor_tensor(out=b[:], in0=mu[:], in1=a[:], op=ALU.mult)
            nc.vector.tensor_tensor(out=b[:], in0=beta[:], in1=b[:],
                                    op=ALU.subtract)
            return a, b

        # ======================= the 5 GIN layers ============================
        for li in range(5):
            src_tab = hfull[li]

            # ---- aggregate + z = h + agg (in place in hT) -------------------
            for b in range(NB):
                idxb = small_p.tile([128, IDXW], I16, tag="idxb")
                nc.sync.dma_start(out=idxb[:],
                                  in_=idx_d[:, b * IDXW:(b + 1) * IDXW])
                gt = gat_p.tile([128, NCH_B, D], F16, tag="gt")
                for ck in range(cfg.NCHK):
                    r0 = ck * cfg.CHR
                    r1 = min(r0 + cfg.CHR, NP)
                    nc.gpsimd.dma_gather(
                        out_ap=gt[:, ck * NCH_BC:(ck + 1) * NCH_BC, :],
                        in_ap=src_tab[r0:r1, :],
                        idxs_ap=idxb[:, ck * 8 * NCH_BC:(ck + 1) * 8 * NCH_BC],
                        num_idxs=NCH_BC * 128,
                        num_idxs_reg=NCH_BC * 128,
                        elem_size=D, single_packet=False)
                S = s_p.tile([128, NCH_B, 128], F16, tag="S")
                i0 = iota128_t[:]
                in0 = bass.AP(i0.tensor, i0.offset,
                              [i0.ap[0], [0, NCH_B], i0.ap[1]])
                i1 = ld_t[:, b * NCH_B:(b + 1) * NCH_B]
                in1 = bass.AP(i1.tensor, i1.offset, [*i1.ap, [0, 128]])
                nc.vector.tensor_tensor(out=S[:], in0=in0, in1=in1,
                                        op=ALU.is_equal)
                agg = ps_agg.tile([D, 128], F32, space="PSUM", tag="agg")
                for j in range(NCH_B):
                    nc.tensor.matmul(out=agg[:], lhsT=gt[:, j, :],
                                     rhs=S[:, j, :], start=(j == 0),
                                     stop=(j == NCH_B - 1))
                bs = slice(b * 128, (b + 1) * 128)
                nc.vector.tensor_tensor(out=hT[:, bs], in0=hT[:, bs],
                                        in1=agg[:], op=ALU.add)

            if stop_stage <= 1:
                break
            # ---- MLP: z1 = relu(W1^T z), z2 = relu(W2^T z1) (in place) ------
            sums = small_p.tile([128, cfg.NGRP], F32, tag="sums")
            ssqs = small_p.tile([128, cfg.NGRP], F32, tag="ssqs")
            for g in range(cfg.NGRP):
                c0 = g * GRP_W
                gw = min(GRP_W, SHP - c0)
                rw = max(0, min(gw, cfg.SH - c0))  # real (non-pad) columns
                gs = slice(c0, c0 + gw)
                p1 = ps_mlp.tile([D, GRP_W], F32, space="PSUM", tag="p1")
                nc.tensor.matmul(out=p1[:, :gw], lhsT=w1_t[li][:],
                                 rhs=hT[:, gs], start=True, stop=True)
                z1 = small_p.tile([D, GRP_W], F32, tag="z1")
                nc.scalar.activation(out=z1[:, :gw], in_=p1[:, :gw],
                                     func=AF.Relu, bias=b1_t[li][:])
                p2 = ps_mlp.tile([D, GRP_W], F32, space="PSUM", tag="p2")
                nc.tensor.matmul(out=p2[:, :gw], lhsT=w2_t[li][:],
                                 rhs=z1[:, :gw], start=True, stop=True)
                nc.scalar.activation(out=hT[:, gs], in_=p2[:, :gw],
                                     func=AF.Relu, bias=b2_t[li][:])
                if rw > 0:
                    rs = slice(c0, c0 + rw)
                    nc.vector.tensor_reduce(out=sums[:, g:g + 1],
                                            in_=hT[:, rs],
                                            axis=mybir.AxisListType.X,
                                            op=ALU.add)
                    sq = small_p.tile([D, GRP_W], F32, tag="sq")
                    nc.vector.tensor_tensor(out=sq[:, :rw], in0=hT[:, rs],
                                            in1=hT[:, rs], op=ALU.mult)
                    nc.vector.tensor_reduce(out=ssqs[:, g:g + 1],
                                            in_=sq[:, :rw],
                                            axis=mybir.AxisListType.X,
                                            op=ALU.add)
                else:
                    nc.gpsimd.memset(sums[:, g:g + 1], 0.0)
                    nc.gpsimd.memset(ssqs[:, g:g + 1], 0.0)

            if stop_stage <= 2:
                break
            # ---- BatchNorm over all N nodes (cross-core stats) --------------
            stat = small_p.tile([128, 2], F32, tag="stat")
            nc.vector.tensor_reduce(out=stat[:, 0:1], in_=sums[:],
                                    axis=mybir.AxisListType.X, op=ALU.add)
            nc.vector.tensor_reduce(out=stat[:, 1:2], in_=ssqs[:],
                                    axis=mybir.AxisListType.X, op=ALU.add)
            red = bn_stats_allreduce(stat, li)
            a, bvec = bn_affine(red, cfg.N, gam_t[li], bet_t[li], f"c{li}")
            nc.scalar.activation(out=hT[:], in_=hT[:], func=AF.Identity,
                                 scale=a[:], bias=bvec[:])

            if stop_stage <= 3:
                break
            # ---- transpose to rows (fp16), pool matmuls, shard DMA ----------
            if li < 4:
                shard_rows = dram.tile([SHP, D], F16, tag=f"rows_{li}")
            pool_ps = ps_pool.tile([D, G], F32, space="PSUM", tag="poolps")
            for b in range(NB):
                bs = slice(b * 128, (b + 1) * 128)
                trp = ps_tr.tile([128, 128], F32, space="PSUM", tag="trp")
                nc.tensor.transpose(out=trp[:], in_=hT[:, bs],
                                    identity=ident_t[:])
                rows = small_p.tile([128, 128], F16, tag="rows")
                nc.scalar.activation(out=rows[:], in_=trp[:], func=AF.Identity)
                if li < 4:
                    nc.sync.dma_start(out=shard_rows[bs, :], in_=rows[:])
                oh = small_p.tile([128, G], F16, tag="oh")
                nc.vector.tensor_tensor(out=oh[:], in0=iotaG_t[:],
                                        in1=_bcast(batchv_t[:, b:b + 1], G),
                                        op=ALU.is_equal)
                nc.tensor.matmul(out=pool_ps[:], lhsT=rows[:], rhs=oh[:],
                                 start=(b == 0), stop=(b == NB - 1))

            if li < 4:
                nc.gpsimd.collective_compute(
                    "AllGather", ALU.bypass, replica_groups=rg,
                    ins=[shard_rows[:]], outs=[hfull[li + 1][:]])

            if stop_stage <= 4:
                break
            # ---- all-reduce pooled sums -------------------------------------
            pool_sb = small_p.tile([D, G], F32, tag="pool_sb")
            nc.scalar.activation(out=pool_sb[:], in_=pool_ps[:],
                                 func=AF.Identity)
            pl_in = dram.tile([D, G], F32, tag=f"pl_in_{li}")
            pl_out = dram.tile([D, G], F32, tag=f"pl_out_{li}",
                               addr_space="Shared")
            nc.sync.dma_start(out=pl_in[:], in_=pool_sb[:])
            nc.gpsimd.collective_compute(
                "AllReduce", ALU.add, replica_groups=rg,
                ins=[pl_in[:]], outs=[pl_out[:]])
            poolT = small_p.tile([D, G], F32, tag="poolT")
            nc.sync.dma_start(out=poolT[:], in_=pl_out[:])

            if stop_stage <= 5:
                break
            # ---- g = fc2(g + pool) (replicated on all cores) ----------------
            zg = small_p.tile([D, G], F32, tag="zg")
            nc.vector.tensor_tensor(out=zg[:], in0=gT[:], in1=poolT[:],
                                    op=ALU.add)
            pg = ps_mlp.tile([D, G], F32, space="PSUM", tag="p1")
            nc.tensor.matmul(out=pg[:], lhsT=fc2w_t[:], rhs=zg[:],
                             start=True, stop=True)
            g2 = small_p.tile([D, G], F32, tag="g2")
            gstat = small_p.tile([128, 2], F32, tag="gstat")
            nc.scalar.activation(out=g2[:], in_=pg[:], func=AF.Relu,
                                 bias=fc2b_t[:], accum_out=gstat[:, 0:1])
            gsq = small_p.tile([D, G], F32, tag="gsq")
            nc.vector.tensor_tensor(out=gsq[:], in0=g2[:], in1=g2[:],
                                    op=ALU.mult)
            nc.vector.tensor_reduce(out=gstat[:, 1:2], in_=gsq[:],
                                    axis=mybir.AxisListType.X, op=ALU.add)
            ga, gb = bn_affine(gstat, cfg.G, fc2g_t, fc2be_t, f"g{li}")
            nc.scalar.activation(out=gT[:], in_=g2[:], func=AF.Identity,
                                 scale=ga[:], bias=gb[:])
            nc.vector.tensor_tensor(out=totT[:], in0=totT[:], in1=gT[:],
                                    op=ALU.add)

        nc.sync.dma_start(out=dbg_d[:, :], in_=hT[:, 0:256])
        # ================== final classifier + log_softmax ===================
        lg_ps = ps_mlp.tile([C, G], F32, space="PSUM", tag="p2")
        nc.tensor.matmul(out=lg_ps[:], lhsT=linw_t[:], rhs=totT[:],
                         start=True, stop=True)
        lg = small_p.tile([C, G], F32, tag="lg")
        nc.scalar.activation(out=lg[:], in_=lg_ps[:], func=AF.Identity,
                             bias=linb_t[:])
        # transpose [C, G] -> [G, C] in 128-graph tiles
        sm = sb.tile([128, cfg.GT, C], F32, tag="sm")
        if G % 128 != 0:
            nc.gpsimd.memset(sm[:], 0.0)
        for t in range(cfg.GT):
            tw = min(128, G - t * 128)
            tp = ps_tr.tile([128, C], F32, space="PSUM", tag="trp")
            nc.tensor.matmul(out=tp[:tw, :], lhsT=lg[:, t * 128:t * 128 + tw],
                             rhs=ident_t[:C, :C], start=True, stop=True,
                             is_transpose=True)
            nc.scalar.activation(out=sm[:tw, t, :], in_=tp[:tw, :],
                                 func=AF.Identity)
        mx = small_p.tile([128, cfg.GT], F32, tag="mx")
        nc.vector.tensor_reduce(out=mx[:], in_=sm[:],
                                axis=mybir.AxisListType.X, op=ALU.max)
        m1 = mx[:]
        nc.vector.tensor_tensor(
            out=sm[:], in0=sm[:],
            in1=bass.AP(m1.tensor, m1.offset, [*m1.ap, [0, C]]),
            op=ALU.subtract)
        ex = small_p.tile([128, cfg.GT, C], F32, tag="ex")
        nc.scalar.activation(out=ex[:], in_=sm[:], func=AF.Exp)
        se = small_p.tile([128, cfg.GT], F32, tag="se")
        nc.vector.tensor_reduce(out=se[:], in_=ex[:],
                                axis=mybir.AxisListType.X, op=ALU.add)
        lse = small_p.tile([128, cfg.GT], F32, tag="lse")
        nc.scalar.activation(out=lse[:], in_=se[:], func=AF.Ln)
        l1 = lse[:]
        nc.vector.tensor_tensor(
            out=sm[:], in0=sm[:],
            in1=bass.AP(l1.tensor, l1.offset, [*l1.ap, [0, C]]),
            op=ALU.subtract)
        for t in range(cfg.GT):
            tw = min(128, G - t * 128)
            nc.sync.dma_start(out=out_d[t * 128:t * 128 + tw, :],
                              in_=sm[:tw, t, :])

    nc.compile()
    return nc


# =========================== host preprocessing ==============================

def preprocess(cfg, x, edge_index, batch, fc1_W, fc1_b, fc1_gamma, fc1_beta):
    """Build all per-core device inputs from the raw graph."""
    N, G, SH, SHP, NB, NCH_B = cfg.N, cfg.G, cfg.SH, cfg.SHP, cfg.NB, cfg.NCH_B
    NCH_BC = cfg.NCH_BC
    src = np.asarray(edge_index[0], np.int64)
    dst = np.asarray(edge_index[1], np.int64)
    batch = np.asarray(batch, np.int64)
    x = np.asarray(x, np.float32)

    # padded global row ids for gather sources
    src_pad = ((src // SH) * SHP + (src % SH)).astype(np.int32)

    # fp16 row-major padded x for layer-1 gathers
    x16 = np.zeros((cfg.NP, D), np.float16)
    for c in range(NCORES):
        x16[c * SHP:c * SHP + SH] = x[c * SH:(c + 1) * SH].astype(np.float16)

    # host-side g0 = _fc(pool(x), fc1)
    pool_x = np.zeros((G, D), np.float32)
    np.add.at(pool_x, batch, x)
    h = np.maximum(pool_x @ np.asarray(fc1_W, np.float32)
                   + np.asarray(fc1_b, np.float32), 0.0)
    mu, v = h.mean(0), h.var(0)
    g0 = ((h - mu) / np.sqrt(v + BN_EPS) * np.asarray(fc1_gamma, np.float32)
          + np.asarray(fc1_beta, np.float32))
    g0T = np.ascontiguousarray(g0.T)  # [D, G]

    iota128 = np.broadcast_to(np.arange(128, dtype=np.float16),
                              (128, 128)).copy()
    iotaG = np.broadcast_to(np.arange(G, dtype=np.float16), (128, G)).copy()

    per_core = []
    for c in range(NCORES):
        lo, hi = c * SH, (c + 1) * SH
        m = (dst >= lo) & (dst < hi)
        s_c = src_pad[m].astype(np.int64)
        d_c = (dst[m] - lo).astype(np.int64)
        blk = d_c >> 7
        chk = s_c // cfg.CHR
        grp = blk * cfg.NCHK + chk                   # (block, chunk) group id
        order = np.argsort(grp, kind="stable")
        s_c, d_c, grp = s_c[order], d_c[order], grp[order]
        ngrp = NB * cfg.NCHK
        counts = np.bincount(grp, minlength=ngrp)
        assert counts.max() <= NCH_BC * 128, (c, counts.max())
        starts = np.zeros(ngrp, np.int64)
        starts[1:] = np.cumsum(counts)[:-1]
        rank = np.arange(len(d_c)) - starts[grp]

        # edge position: group g occupies 128-edge chunks
        # [g*NCH_BC, (g+1)*NCH_BC); within: chunk j lane p
        e_chunk = grp * NCH_BC + (rank >> 7)         # global chunk index
        lane = rank & 127

        # one-hot slot ids, aligned with chunk columns
        ld = np.full((128, NB * cfg.NCH_B), PAD_SLOT, np.float16)
        ld[lane, e_chunk] = (d_c & 127).astype(np.float16)

        # int16 gather indices (chunk-relative), wrapped into 16 partitions
        # and replicated 8x: index i of a 128-idx run at [i%16, i//16]
        idx16 = np.zeros((16, NB * cfg.NCH_B * 8), np.int16)
        loc = (s_c % cfg.CHR).astype(np.int16)
        icol = e_chunk * 8 + ((rank & 127) >> 4)
        irow = rank & 15
        idx16[irow, icol] = loc
        idx16 = np.tile(idx16, (8, 1))

        bv = np.full((128, NB), PAD_GRAPH, np.float16)
        nloc = np.arange(SH)
        bv[nloc & 127, nloc >> 7] = batch[lo:hi].astype(np.float16)

        xT = np.zeros((D, SHP), np.float32)
        xT[:, :SH] = x[lo:hi].T

        per_core.append(dict(xT=xT, idx16=idx16, ld=ld, batchv=bv))

    shared = dict(x16=x16, g0T=g0T, iota128=iota128, iotaG=iotaG)
    return per_core, shared


def make_in_maps(cfg, inputs):
    per_core, shared = preprocess(
        cfg, inputs["x"], inputs["edge_index"], inputs["batch"],
        inputs["fc1_W"], inputs["fc1_b"], inputs["fc1_gamma"],
        inputs["fc1_beta"])
    weights = {k: np.asarray(inputs[k], np.float32) for k in
               ("conv_W1", "conv_W2", "conv_b1", "conv_b2", "conv_gamma",
                "conv_beta", "fc2_W", "fc2_b", "fc2_gamma", "fc2_beta",
                "lin_W", "lin_b")}
    in_maps = []
    for c in range(NCORES):
        m = dict(per_core[c])
        m.update(shared)
        m.update(weights)
        in_maps.append(m)
    return in_maps


def compute_nch_b(cfg_n, edge_index):
    """Smallest per-(dst-block, table-chunk) budget that fits the graph."""
    src = np.asarray(edge_index[0], np.int64)
    dst = np.asarray(edge_index[1], np.int64)
    SH, SHP = cfg_n.SH, cfg_n.SHP
    d_loc = dst % SH
    src_pad = (src // SH) * SHP + (src % SH)
    blk_g = (dst // SH) * cfg_n.NB + (d_loc >> 7)
    grp = blk_g * cfg_n.NCHK + src_pad // cfg_n.CHR
    counts = np.bincount(grp, minlength=NCORES * cfg_n.NB * cfg_n.NCHK)
    return int((counts.max() + 127) // 128)


def make_cfg(inputs, G):
    x = np.asarray(inputs["x"])
    e = np.asarray(inputs["edge_index"])
    N, E = x.shape[0], e.shape[1]
    pre = Cfg(N, E, G, 1)
    return Cfg(N, E, G, compute_nch_b(pre, e))


def run(inputs, G, trace=False, trace_kwargs=None):
    cfg = make_cfg(inputs, G)
    nc = build_program(cfg)
    in_maps = make_in_maps(cfg, inputs)
    res = bass_utils.run_bass_kernel_spmd(
        nc, in_maps, core_ids=list(range(NCORES)), trace=trace,
        **(trace_kwargs or {}))
    return res.results[0]["out"], res


def kernel(**inputs) -> np.ndarray:
    out, _ = run(inputs, G=512, trace=False)
    return out
